# revision 1
# baseline (speedup 1.0000x reference)
"""Trainium2 Bass kernel for nn_EncoderBlock (pre-norm transformer encoder block).

Sharding (8 cores, zero collectives):
  core c -> batch b = c//4, query-row block r = (c%4)*1024 .. +1024.
  Each core redundantly computes K/V for its batch, but ONLY over the keys the
  attention mask keeps (mask==0 keys contribute exp(-1e9)=0 in the reference,
  so they are dropped on the host and the kernel never sees them).

Per-core pipeline (all matmuls bf16, statistics/residuals fp32):
  norm1 -> (DMA-transpose via DRAM, band-pipelined) -> Q^T/K^T/V projections
  scores^T = K^T.T @ Q^T (head pairs packed in PE row groups, K=64 each)
  P^T = exp(scores/8 + padbias) on ScalarE (pad keys get -30 bias -> exp ~ 0)
  ctx^T accumulation: V matmul + concurrent ones-column matmul (PE col groups)
    -> softmax denominators ride along for free; scores/exp software-pipelined
    one step ahead of the ctx matmuls so PE and ACT overlap
  divide, W_O matmul + residual + norm2 interleaved per q-chunk, FFN last.
"""

import math
from contextlib import ExitStack

import ml_dtypes
import numpy as np

B, S, D = 2, 4096, 768
H, DK, DFF = 12, 64, 3072
KD = D // 128        # 6 k-tiles over d_model
FT = DFF // 128      # 24 tiles over d_ff
Q = 1024             # query rows per core
QT = Q // 128        # 8 query sub-tiles
QC = 2               # q chunks of 512
NCORES = 8
EPS = 1e-6
VAR_SCALE = float(D) / float(D - 1)  # torch.std is unbiased (ddof=1)
BAND = 4
PE_T_BANDS = 3       # first N bands transpose on PE instead of DMA


def _bands(ntiles, band):
    out = []
    t = 0
    while t < ntiles:
        out.append((t, min(band, ntiles - t)))
        t += band
    return out


def _build(KT, SAFE=None):
    import concourse.bass as bass
    import concourse.mybir as mybir
    import concourse.tile as tile
    from concourse import bacc
    from concourse.bass import ds, ts

    NK = KT * 128
    if SAFE is None:
        SAFE = KT - 2   # tiles < SAFE are guaranteed all-kept (pads are a suffix)
    f32 = mybir.dt.float32
    bf16 = mybir.dt.bfloat16
    i32 = mybir.dt.int32
    AF = mybir.ActivationFunctionType
    OP = mybir.AluOpType

    nc = bacc.Bacc()

    xq_d = nc.dram_tensor("xq", [Q, D], f32, kind="ExternalInput")
    xk_d = nc.dram_tensor("xk", [NK, D], bf16, kind="ExternalInput")
    km_d = nc.dram_tensor("kmask", [NK], i32, kind="ExternalInput")
    wq_d = nc.dram_tensor("wqT", [D, D], bf16, kind="ExternalInput")
    wk_d = nc.dram_tensor("wkT", [D, D], bf16, kind="ExternalInput")
    wv_d = nc.dram_tensor("wvT", [D, D], bf16, kind="ExternalInput")
    wo_d = nc.dram_tensor("woT", [D, D], bf16, kind="ExternalInput")
    w1_d = nc.dram_tensor("w1T", [D, DFF], bf16, kind="ExternalInput")
    w2_d = nc.dram_tensor("w2T", [DFF, D], bf16, kind="ExternalInput")
    bq_d = nc.dram_tensor("bq", [D], f32, kind="ExternalInput")
    bk_d = nc.dram_tensor("bk", [D], f32, kind="ExternalInput")
    bv_d = nc.dram_tensor("bv", [D], f32, kind="ExternalInput")
    bo_d = nc.dram_tensor("bo", [D], f32, kind="ExternalInput")
    b1_d = nc.dram_tensor("b1", [DFF], f32, kind="ExternalInput")
    b2_d = nc.dram_tensor("b2", [D], f32, kind="ExternalInput")
    a1_d = nc.dram_tensor("a1", [1], f32, kind="ExternalInput")
    g1_d = nc.dram_tensor("g1", [1], f32, kind="ExternalInput")
    a2_d = nc.dram_tensor("a2", [1], f32, kind="ExternalInput")
    g2_d = nc.dram_tensor("g2", [1], f32, kind="ExternalInput")
    out_d = nc.dram_tensor("out", [Q, D], f32, kind="ExternalOutput")

    def norm_tile(spool, xt, a_b, g_b, out_t):
        # out = alpha * (x - mean) / (std_unbiased + eps) + beta, reduced over D
        st = spool.tile([128, 3, 6], f32, tag="bnst")
        for g in range(3):
            nc.vector.bn_stats(st[:, g, :], xt[:, ts(g, 256)])
        mv = spool.tile([128, 2], f32, tag="bnmv")
        nc.vector.bn_aggr(mv, st)
        rp = spool.tile([128, 1], f32, tag="rp")
        nc.scalar.activation(rp, mv[:, 1:2], AF.Sqrt, bias=0.0, scale=VAR_SCALE)
        nc.vector.tensor_scalar_add(rp, rp, EPS)
        nc.vector.reciprocal(rp, rp)
        nc.vector.tensor_tensor(rp, rp, a_b, OP.mult)
        cb = spool.tile([128, 1], f32, tag="cb")
        nc.vector.tensor_tensor(cb, mv[:, 0:1], rp, OP.mult)
        nc.vector.tensor_tensor(cb, g_b, cb, OP.subtract)
        nc.vector.tensor_scalar(out_t, xt, rp, cb, OP.mult, OP.add)

    with tile.TileContext(nc) as tc, ExitStack() as ctx:
        const = ctx.enter_context(tc.tile_pool(name="const", bufs=1))
        dram = ctx.enter_context(tc.tile_pool(name="dram", bufs=1, space="DRAM"))

        # --- broadcast scalars alpha/beta -> [128,1]
        scal = {}
        for name, d_t in (("a1", a1_d), ("g1", g1_d), ("a2", a2_d), ("g2", g2_d)):
            t = const.tile([128, 1], f32, tag=f"sc_{name}")
            nc.gpsimd.dma_start(out=t, in_=d_t[:].to_broadcast((128, 1)))
            scal[name] = t

        # --- per-partition bias stripes
        bqp = const.tile([128, KD], f32, tag="bqp")
        bkp = const.tile([128, KD], f32, tag="bkp")
        b1p = const.tile([128, FT], f32, tag="b1p")

        # --- free-dim biases broadcast [D] -> [128, D] (step-0 partition on a
        # DRAM source is legal)
        ones_col = const.tile([128, 1], bf16, tag="ones_col")
        nc.vector.memset(ones_col, 1.0)
        # warm the sqrt activation table before the first norm needs it
        warmt = const.tile([128, 1], f32, tag="warmt")
        nc.vector.memset(warmt, 1.0)
        nc.scalar.activation(warmt, warmt, AF.Sqrt, bias=0.0, scale=1.0)
        ident = const.tile([128, 128], bf16, tag="ident")
        from concourse.masks import make_identity
        make_identity(nc, ident)

        bvb = const.tile([128, D], f32, tag="bvb")
        bob = const.tile([128, D], f32, tag="bob")
        b2b = const.tile([128, D], f32, tag="b2b")
        for d_t, dst in ((bv_d, bvb), (bo_d, bob), (b2_d, b2b)):
            src = d_t[:]
            bcast = bass.AP(tensor=src.tensor, offset=src.offset,
                            ap=[[0, 128], [1, D]])
            nc.gpsimd.dma_start(out=dst, in_=bcast)

        # --- pad-mask bias: (mask-1)*30 -> 0 for kept keys, -30 for pads
        kmi = const.tile([128, KT], i32, tag="kmi")
        kmf = const.tile([128, KT], f32, tag="kmf")
        padb = const.tile([128, KT], f32, tag="padb")

        # --- long-lived activations (pool releases are LIFO-stacked)
        cTp = ctx.enter_context(tc.tile_pool(name="cTp", bufs=1))
        cT = cTp.tile([128, KD, Q], bf16, tag="cT")
        woTp = ctx.enter_context(tc.tile_pool(name="woTp", bufs=1))
        woT_sb = woTp.tile([128, KD, D], bf16, tag="woT")
        x1p = ctx.enter_context(tc.tile_pool(name="x1p", bufs=1))
        x1 = x1p.tile([128, QT, D], f32, tag="x1")
        h2Tp = ctx.enter_context(tc.tile_pool(name="h2Tp", bufs=1))
        h2T = h2Tp.tile([128, KD, Q], bf16, tag="h2T")
        # first third of W1, loaded during attention so ff1 starts immediately
        w1ap = ctx.enter_context(tc.tile_pool(name="w1ap", bufs=1))
        w1a = w1ap.tile([128, KD, 1024], bf16, tag="w1a")

        qkv_cm = tc.tile_pool(name="qkvp", bufs=1)   # lives A..D
        qkvp = qkv_cm.__enter__()
        kT = qkvp.tile([128, KD, NK], bf16, tag="kT")
        qT = qkvp.tile([128, KD, Q], bf16, tag="qT")
        vv = qkvp.tile([128, KT, D], bf16, tag="vv")

        hk_dram = dram.tile([NK, D], bf16, tag="hk_dram")
        hq_dram = dram.tile([Q, D], bf16, tag="hq_dram")
        h2_dram = dram.tile([Q, D], bf16, tag="h2_dram")

        # ========== Phase A/B/C: norm1 + transpose + QKV, band-pipelined =====
        with tc.tile_pool(name="normA", bufs=2) as npool, \
             tc.tile_pool(name="xtp", bufs=4) as xtp, \
             tc.tile_pool(name="stats", bufs=8) as spool, \
             tc.tile_pool(name="hband", bufs=2) as hbp, \
             tc.tile_pool(name="wqkv", bufs=1) as wp, \
             tc.tile_pool(name="psC", bufs=4, space="PSUM") as pp:
            # issue band-0 x loads before the weight DMAs so the first norms
            # are not queued behind 3.5MB of weights
            b0 = []
            t0_, nt_ = _bands(KT, BAND)[0]
            for t in range(t0_, t0_ + nt_):
                xt = xtp.tile([128, D], bf16, tag="xt")
                nc.sync.dma_start(xt, xk_d[ts(t, 128), :])
                b0.append(xt)

            wqs = wp.tile([128, KD, D], bf16, tag="wqs")
            wks = wp.tile([128, KD, D], bf16, tag="wks")
            wvs = wp.tile([128, KD, D], bf16, tag="wvs")
            # load order matches the band schedule: K0 needs wk, then V, then Q0
            for k in range(KD):
                nc.sync.dma_start(wks[:, k, :], wk_d[ts(k, 128), :])
            for k in range(KD):
                nc.sync.dma_start(wqs[:, k, :], wq_d[ts(k, 128), :])
            nc.sync.dma_start(bqp, bq_d[:].rearrange("(o p) -> p o", p=128))
            nc.sync.dma_start(bkp, bk_d[:].rearrange("(o p) -> p o", p=128))
            nc.sync.dma_start(b1p, b1_d[:].rearrange("(o p) -> p o", p=128))
            nc.sync.dma_start(kmi, km_d[:].rearrange("(t p) -> p t", p=128))
            nc.vector.tensor_copy(out=kmf, in_=kmi)
            nc.vector.tensor_scalar(padb, kmf, 1.0, 30.0, OP.subtract, OP.mult)
            for k in range(KD):
                nc.sync.dma_start(woT_sb[:, k, :], wo_d[ts(k, 128), :])
            for k in range(KD):
                nc.sync.dma_start(wvs[:, k, :], wv_d[ts(k, 128), :])
            

            # Band schedule interleaves K-side and Q-side so PE always has
            # projection matmuls while the next band's norm/transpose runs.
            # Band 0 transposes on the (idle) PE instead of the DMA round trip.
            kb = [("k", t0, nt) for t0, nt in _bands(KT, BAND)]
            qb = [("q", t0, nt) for t0, nt in _bands(QT, BAND)]
            sched = [kb[0], qb[0]] + kb[1:3] + [qb[1]] + kb[3:]

            for bi, (side, t0, nt) in enumerate(sched):
                w = nt * 128
                hb = hbp.tile([128, KD, 512], bf16, tag="hb")
                x_d, h_dram = (xk_d, hk_dram) if side == "k" else (xq_d, hq_dram)
                for t in range(t0, t0 + nt):
                    if bi == 0:
                        xt = b0[t - t0]
                    else:
                        xt = xtp.tile([128, D], f32 if side == "q" else bf16,
                                      tag="xtq" if side == "q" else "xt")
                        nc.sync.dma_start(xt, x_d[ts(t, 128), :])
                    ht = npool.tile([128, D], bf16, tag="ht")
                    norm_tile(spool, xt, scal["a1"], scal["g1"], ht)
                    if bi < PE_T_BANDS:
                        for j in range(KD):
                            pst = pp.tile([128, 128], bf16, tag="pst",
                                          space="PSUM")
                            nc.tensor.transpose(pst, ht[:, ts(j, 128)], ident)
                            nc.vector.tensor_copy(
                                out=hb[:, j, ds((t - t0) * 128, 128)], in_=pst)
                    else:
                        nc.sync.dma_start(h_dram[ts(t, 128), :], ht)
                if bi >= PE_T_BANDS:
                    for j in range(KD):
                        nc.sync.dma_start_transpose(
                            hb[:, j, :w], h_dram[ds(t0 * 128, w), ts(j, 128)])

                if side == "k":
                    # K^T for this band
                    for j in range(KD):
                        ps = pp.tile([128, 512], f32, tag="psc")
                        for k in range(KD):
                            nc.tensor.matmul(ps[:, :w], wks[:, k, ts(j, 128)],
                                             hb[:, k, :w],
                                             start=(k == 0), stop=(k == KD - 1))
                        nc.vector.tensor_scalar_add(kT[:, j, ds(t0 * 128, w)],
                                                    ps[:, :w], bkp[:, j:j + 1])
                    # V for this band
                    for t in range(t0, t0 + nt):
                        loc = (t - t0) * 128
                        for hh in range(2):
                            ps = pp.tile([128, 512], f32, tag="psc")
                            for k in range(KD):
                                nc.tensor.matmul(ps[:, :384],
                                                 hb[:, k, ds(loc, 128)],
                                                 wvs[:, k, ts(hh, 384)],
                                                 start=(k == 0),
                                                 stop=(k == KD - 1))
                            nc.vector.tensor_tensor(vv[:, t, ts(hh, 384)],
                                                    ps[:, :384],
                                                    bvb[:, ts(hh, 384)], OP.add)
                else:
                    for j in range(KD):
                        ps = pp.tile([128, 512], f32, tag="psc")
                        for k in range(KD):
                            nc.tensor.matmul(ps[:, :w], wqs[:, k, ts(j, 128)],
                                             hb[:, k, :w],
                                             start=(k == 0), stop=(k == KD - 1))
                        nc.vector.tensor_scalar_add(qT[:, j, ds(t0 * 128, w)],
                                                    ps[:, :w], bqp[:, j:j + 1])

        # ========== Phase D/E/F: attention + W_O + norm2, per q-chunk ========
        psD_cm = tc.tile_pool(name="psD", bufs=2, space="PSUM")
        psS = psD_cm.__enter__()
        psCx_cm = tc.tile_pool(name="psCx", bufs=4, space="PSUM")
        psCx = psCx_cm.__enter__()
        with tc.tile_pool(name="ptp", bufs=4) as ptp, \
             tc.tile_pool(name="asm", bufs=2) as asm, \
             tc.tile_pool(name="normB", bufs=3) as npool2, \
             tc.tile_pool(name="stats2", bufs=8) as spool2, \
             tc.tile_pool(name="xqb", bufs=4) as xqbp:

            def scores_exp(c, p, kt):
                pss = psS.tile([128, 1024], f32, tag="pss")
                nc.tensor.matmul(pss[:, 0:512], kT[0:64, p, ts(kt, 128)],
                                 qT[0:64, p, ts(c, 512)], start=True, stop=True)
                nc.tensor.matmul(pss[:, 512:1024], kT[64:128, p, ts(kt, 128)],
                                 qT[64:128, p, ts(c, 512)], start=True, stop=True)
                pt = ptp.tile([128, 1024], bf16, tag="pt")
                nc.scalar.activation(pt, pss, AF.Exp,
                                     bias=padb[:, kt:kt + 1], scale=0.125)
                return pt

            def divide_out(c, p, pc0, pc1):
                for hh, pc in ((0, pc0), (1, pc1)):
                    rr = asm.tile([1, 512], f32, tag="rr")
                    nc.vector.reciprocal(rr, pc[64:65, :])
                    rb = asm.tile([64, 512], f32, tag="rb")
                    nc.gpsimd.partition_broadcast(rb, rr)
                    nc.vector.tensor_tensor(cT[ds(hh * 64, 64), p, ts(c, 512)],
                                            pc[0:64, :], rb, OP.mult)

            pending = None
            for c in range(QC):
                # residual tiles for this chunk's W_O, loaded under attention
                xbs = []
                for st_ in range(4):
                    g = c * 4 + st_
                    xb = xqbp.tile([128, D], f32, tag="xb")
                    nc.sync.dma_start(xb, xq_d[ts(g, 128), :])
                    nc.vector.tensor_tensor(xb, xb, bob, OP.add)
                    xbs.append(xb)
                for p in range(KD):
                    pc0 = psCx.tile([128, 512], f32, tag="pc")
                    pc1 = psCx.tile([128, 512], f32, tag="pc")
                    pt_next = scores_exp(c, p, 0)
                    for kt in range(KT):
                        pt = pt_next
                        if kt + 1 < KT:
                            pt_next = scores_exp(c, p, kt + 1)
                        st, sp = (kt == 0), (kt == KT - 1)
                        nc.tensor.matmul(pc0[0:64, :],
                                         vv[:, kt, ds(2 * p * 64, 64)],
                                         pt[:, 0:512], start=st, stop=sp)
                        nc.tensor.matmul(pc0[64:65, :], ones_col,
                                         pt[:, 0:512], start=st, stop=sp)
                        nc.tensor.matmul(pc1[0:64, :],
                                         vv[:, kt, ds((2 * p + 1) * 64, 64)],
                                         pt[:, 512:1024], start=st, stop=sp)
                        nc.tensor.matmul(pc1[64:65, :], ones_col,
                                         pt[:, 512:1024], start=st, stop=sp)
                    if pending is not None:
                        divide_out(*pending)
                    pending = (c, p, pc0, pc1)

                divide_out(*pending)
                pending = None

                # W_O + residual for this chunk (psum slots shared with psCx)
                for st_ in range(4):
                    g = c * 4 + st_
                    xb = xbs[st_]
                    for hh in range(2):
                        ps = psCx.tile([128, 512], f32, tag="pc")
                        for j in range(KD):
                            nc.tensor.matmul(ps[:, :384], cT[:, j, ts(g, 128)],
                                             woT_sb[:, j, ts(hh, 384)],
                                             start=(j == 0), stop=(j == KD - 1))
                        nc.vector.tensor_tensor(x1[:, g, ts(hh, 384)],
                                                ps[:, :384], xb[:, ts(hh, 384)],
                                                OP.add)

                # norm2 + transpose for this chunk
                for st_ in range(4):
                    g = c * 4 + st_
                    ht = npool2.tile([128, D], bf16, tag="h2t")
                    norm_tile(spool2, x1[:, g, :], scal["a2"], scal["g2"], ht)
                    nc.sync.dma_start(h2_dram[ts(g, 128), :], ht)
                for j in range(KD):
                    nc.sync.dma_start_transpose(
                        h2T[:, j, ts(c, 512)],
                        h2_dram[ds(c * 512, 512), ts(j, 128)])

                if c == 0:
                    for k in range(KD):
                        nc.sync.dma_start(w1a[:, k, :],
                                          w1_d[ts(k, 128), 0:1024])

        qkv_cm.__exit__(None, None, None)  # free kT/qT/vv

        # ================= Phase G: FFN + residual =================
        wff = ctx.enter_context(tc.tile_pool(name="wff", bufs=1))
        w1b = wff.tile([128, KD, 2048], bf16, tag="w1b")
        for k in range(KD):
            for h3_ in range(2):
                nc.sync.dma_start(
                    w1b[:, k, ds(h3_ * 1024, 1024)],
                    w1_d[ts(k, 128), ds(1024 + h3_ * 1024, 1024)])
        w2s = wff.tile([128, FT, D], bf16, tag="w2s")
        for k in range(FT):
            nc.sync.dma_start(w2s[:, k, :], w2_d[ts(k, 128), :])

        pg = psCx
        with tc.tile_pool(name="h3p", bufs=1) as h3p, \
             tc.tile_pool(name="outp", bufs=3) as outp:
            for c in range(QC):
                h3 = h3p.tile([128, FT, 512], bf16, tag="h3")
                for f in range(FT):
                    ps = pg.tile([128, 512], f32, tag="pc")
                    for k in range(KD):
                        w1sl = (w1a[:, k, ts(f, 128)] if f < 8 else
                                w1b[:, k, ds((f - 8) * 128, 128)])
                        nc.tensor.matmul(ps, w1sl,
                                         h2T[:, k, ts(c, 512)],
                                         start=(k == 0), stop=(k == KD - 1))
                    nc.scalar.activation(h3[:, f, :], ps, AF.Relu,
                                         bias=b1p[:, f:f + 1], scale=1.0)
                for st_ in range(4):
                    g = c * 4 + st_
                    ot = outp.tile([128, D], f32, tag="ot")
                    for hh in range(2):
                        ps = pg.tile([128, 512], f32, tag="pc")
                        for k in range(FT):
                            nc.tensor.matmul(ps[:, :384], h3[:, k, ts(st_, 128)],
                                             w2s[:, k, ts(hh, 384)],
                                             start=(k == 0), stop=(k == FT - 1))
                        nc.vector.tensor_tensor(ot[:, ts(hh, 384)], ps[:, :384],
                                                x1[:, g, ts(hh, 384)], OP.add)
                    nc.vector.tensor_tensor(ot, ot, b2b, OP.add)
                    nc.sync.dma_start(out_d[ts(g, 128), :], ot)

        psCx_cm.__exit__(None, None, None)
        psD_cm.__exit__(None, None, None)

    nc.finalize()
    return nc


def _prep_inputs(inputs):
    bf = ml_dtypes.bfloat16
    x = np.asarray(inputs["x"], np.float32)
    mask = np.asarray(inputs["mask"], np.int32).reshape(B, S)

    kept = [np.nonzero(mask[b])[0] for b in range(B)]
    nk_max = max(len(kept[0]), len(kept[1]))
    KT = max(2, int(math.ceil(nk_max / 128.0)))
    SAFE = min(len(kept[0]), len(kept[1])) // 128
    NK = KT * 128

    xk = []
    km = []
    for b in range(B):
        n = len(kept[b])
        xkb = np.zeros((NK, D), np.float32)
        xkb[:n] = x[b][kept[b]]
        if n < NK:
            # pad rows get real data (not zeros) so ln(var) in the norm stays
            # finite; their attention contribution is killed by the -30 bias
            xkb[n:] = xkb[0]
        xk.append(np.ascontiguousarray(xkb.astype(bf)))
        kmb = np.zeros(NK, np.int32)
        kmb[:n] = 1
        km.append(kmb)

    def w_t(name):
        return np.ascontiguousarray(
            np.asarray(inputs[name], np.float32).T.astype(bf))

    shared = {
        "wqT": w_t("wq"), "wkT": w_t("wk"), "wvT": w_t("wv"), "woT": w_t("wo"),
        "w1T": w_t("w1"), "w2T": w_t("w2"),
        "bq": np.asarray(inputs["bq"], np.float32),
        "bk": np.asarray(inputs["bk"], np.float32),
        "bv": np.asarray(inputs["bv"], np.float32),
        "bo": np.asarray(inputs["bo"], np.float32),
        "b1": np.asarray(inputs["b1"], np.float32),
        "b2": np.asarray(inputs["b2"], np.float32),
        "a1": np.asarray(inputs["alpha1"], np.float32).reshape(1),
        "g1": np.asarray(inputs["beta1"], np.float32).reshape(1),
        "a2": np.asarray(inputs["alpha2"], np.float32).reshape(1),
        "g2": np.asarray(inputs["beta2"], np.float32).reshape(1),
    }

    in_maps = []
    for c in range(NCORES):
        b, r = c // 4, (c % 4) * Q
        m = dict(shared)
        m["xq"] = np.ascontiguousarray(x[b, r:r + Q])
        m["xk"] = xk[b]
        m["kmask"] = km[b]
        in_maps.append(m)
    return KT, SAFE, in_maps


def kernel(**inputs):
    from concourse.bass_utils import run_bass_kernel_spmd

    KT, SAFE, in_maps = _prep_inputs(inputs)
    nc = _build(KT, SAFE)
    res = run_bass_kernel_spmd(nc, in_maps, core_ids=list(range(NCORES)))
    out = np.empty((B, S, D), np.float32)
    for c in range(NCORES):
        b, r = c // 4, (c % 4) * Q
        out[b, r:r + Q] = res.results[c]["out"]
    return out


if __name__ == "__main__":
    rng = np.random.default_rng(0)
    demo = {
        "x": rng.standard_normal((B, S, D), dtype=np.float32),
        "mask": rng.integers(0, 2, (B, 1, 1, S)).astype(np.int32),
        "wq": rng.standard_normal((D, D), dtype=np.float32) * 0.02,
        "bq": np.zeros(D, np.float32),
        "wk": rng.standard_normal((D, D), dtype=np.float32) * 0.02,
        "bk": np.zeros(D, np.float32),
        "wv": rng.standard_normal((D, D), dtype=np.float32) * 0.02,
        "bv": np.zeros(D, np.float32),
        "wo": rng.standard_normal((D, D), dtype=np.float32) * 0.02,
        "bo": np.zeros(D, np.float32),
        "w1": rng.standard_normal((DFF, D), dtype=np.float32) * 0.02,
        "b1": np.zeros(DFF, np.float32),
        "w2": rng.standard_normal((D, DFF), dtype=np.float32) * 0.02,
        "b2": np.zeros(D, np.float32),
        "alpha1": np.ones(1, np.float32), "beta1": np.ones(1, np.float32),
        "alpha2": np.ones(1, np.float32), "beta2": np.ones(1, np.float32),
    }
    out = kernel(**demo)
    print("out", out.shape, out.dtype, float(np.abs(out).mean()))



# revision 14
# speedup vs baseline: 1.1988x; 1.1988x over previous
"""Trainium2 Bass kernel for nn_EncoderBlock (pre-norm transformer encoder).

Sharding (8 cores, zero collectives): core c -> batch b = c//4, query-row
block r = (c%4)*1024 .. +1024.  Each core redundantly computes K/V for its
batch over ONLY the keys the attention mask keeps (mask==0 keys are dropped
host-side; exp(-1e9) = 0 in the reference so they contribute nothing).

Pipeline (per core):
  norm1 -> PE-transpose -> fp8 DoubleRow Q/K/V projections (weights x16 fp8)
  scores^T = K^T.T @ Q^T in fp8 operands / f32 psum
  P' = exp(scores/8 + padbias + ln PS) -> fp8 (pads underflow to exact 0)
  ctx[q,d] accumulated with queries on PSUM partitions; softmax denominator
    rides as a 65th ones-column of V (fused into the same matmul)
  divide (per-partition scalar) -> PE-transpose ctx -> W_O fp8-DR -> x1
  norm2 -> PE-transpose -> FFN in bf16 (fp8 fails the accuracy budget here);
  chunk-0 FFN is interleaved into chunk-1's attention stream in small slices
  to keep PE dense under the ACT-bound softmax phase.
"""

import math
from contextlib import ExitStack

import ml_dtypes
import numpy as np

B, S, D = 2, 4096, 768
H, DK, DFF = 12, 64, 3072
KD = D // 128         # 6 k-tiles over d_model
KD2 = KD // 2         # 3 DoubleRow steps over d_model
FT = DFF // 128       # 24 tiles over d_ff
Q = 1024              # query rows per core
QT = Q // 128
QC = 2                # q chunks of 512
NCORES = 8
EPS = 1e-6
VAR_SCALE = float(D) / float(D - 1)
BAND = 4
WS = 16.0             # host-side fp8 weight scale (qkv/wo only)
CS = 16.0             # ctx scale in hstage/cT (fp8 range)
PS = 0.25             # P' = PS * softmax numerator (fp8 range, no overflow)
LOG_PS = math.log(PS)

DEBUG = False


def _bands(ntiles, band):
    out = []
    t = 0
    while t < ntiles:
        out.append((t, min(band, ntiles - t)))
        t += band
    return out


def _build(KT, SAFE=None):
    import concourse.bass as bass
    import concourse.mybir as mybir
    import concourse.tile as tile
    from concourse import bacc
    from concourse.bass import ds, ts

    NK = KT * 128
    f32 = mybir.dt.float32
    bf16 = mybir.dt.bfloat16
    fp8 = mybir.dt.float8e4
    i32 = mybir.dt.int32
    AF = mybir.ActivationFunctionType
    OP = mybir.AluOpType
    DR = mybir.MatmulPerfMode.DoubleRow

    nc = bacc.Bacc()

    xq_d = nc.dram_tensor("xq", [Q, D], f32, kind="ExternalInput")
    xk_d = nc.dram_tensor("xk", [NK, D], bf16, kind="ExternalInput")
    km_d = nc.dram_tensor("kmask", [NK], i32, kind="ExternalInput")
    wq_d = nc.dram_tensor("wqT", [D, D], fp8, kind="ExternalInput")
    wk_d = nc.dram_tensor("wkT", [D, D], fp8, kind="ExternalInput")
    wv_d = nc.dram_tensor("wvT", [D, D], fp8, kind="ExternalInput")
    wo_d = nc.dram_tensor("woT", [D, D], fp8, kind="ExternalInput")
    w1_d = nc.dram_tensor("w1T", [D, DFF], bf16, kind="ExternalInput")
    w2_d = nc.dram_tensor("w2T", [DFF, D], bf16, kind="ExternalInput")
    bq_d = nc.dram_tensor("bq", [D], f32, kind="ExternalInput")
    bk_d = nc.dram_tensor("bk", [D], f32, kind="ExternalInput")
    bv_d = nc.dram_tensor("bv16", [D], f32, kind="ExternalInput")
    bo_d = nc.dram_tensor("bo", [D], f32, kind="ExternalInput")
    b1_d = nc.dram_tensor("b1", [DFF], f32, kind="ExternalInput")
    b2_d = nc.dram_tensor("b2", [D], f32, kind="ExternalInput")
    a1_d = nc.dram_tensor("a1", [1], f32, kind="ExternalInput")
    g1_d = nc.dram_tensor("g1", [1], f32, kind="ExternalInput")
    a2_d = nc.dram_tensor("a2", [1], f32, kind="ExternalInput")
    g2_d = nc.dram_tensor("g2", [1], f32, kind="ExternalInput")
    out_d = nc.dram_tensor("out", [Q, D], f32, kind="ExternalOutput")

    def norm_tile(spool, xt, a_b, g_b, out_t):
        # out = alpha * (x - mean) / (std_unbiased + eps) + beta over D
        st = spool.tile([128, 3, 6], f32, tag="bnst")
        for g in range(3):
            nc.vector.bn_stats(st[:, g, :], xt[:, ts(g, 256)])
        mv = spool.tile([128, 2], f32, tag="bnmv")
        nc.vector.bn_aggr(mv, st)
        rp = spool.tile([128, 1], f32, tag="rp")
        nc.scalar.activation(rp, mv[:, 1:2], AF.Sqrt, bias=0.0, scale=VAR_SCALE)
        nc.vector.tensor_scalar_add(rp, rp, EPS)
        nc.vector.reciprocal(rp, rp)
        nc.vector.tensor_tensor(rp, rp, a_b, OP.mult)
        cb = spool.tile([128, 1], f32, tag="cb")
        nc.vector.tensor_tensor(cb, mv[:, 0:1], rp, OP.mult)
        nc.vector.tensor_tensor(cb, g_b, cb, OP.subtract)
        nc.gpsimd.tensor_scalar(out_t, xt, rp, cb, OP.mult, OP.add)

    with tile.TileContext(nc) as tc, ExitStack() as ctx:
        const = ctx.enter_context(tc.tile_pool(name="const", bufs=1))

        scal = {}
        for name, d_t in (("a1", a1_d), ("g1", g1_d), ("a2", a2_d), ("g2", g2_d)):
            t = const.tile([128, 1], f32, tag=f"sc_{name}")
            nc.gpsimd.dma_start(out=t, in_=d_t[:].to_broadcast((128, 1)))
            scal[name] = t

        bqp = const.tile([128, KD], f32, tag="bqp")
        bkp = const.tile([128, KD], f32, tag="bkp")
        b1p = const.tile([128, FT], f32, tag="b1p")

        # warm the sqrt activation table before the first norm needs it
        warmt = const.tile([128, 1], f32, tag="warmt")
        nc.vector.memset(warmt, 1.0)
        nc.scalar.activation(warmt, warmt, AF.Sqrt, bias=0.0, scale=1.0)
        ident = const.tile([128, 128], bf16, tag="ident")
        from concourse.masks import make_identity
        make_identity(nc, ident)

        bvb = const.tile([128, D], f32, tag="bvb")   # 16*bv broadcast
        bob = const.tile([128, D], f32, tag="bob")
        b2b = const.tile([128, D], f32, tag="b2b")
        for d_t, dst in ((bv_d, bvb), (bo_d, bob), (b2_d, b2b)):
            src = d_t[:]
            bcast = bass.AP(tensor=src.tensor, offset=src.offset,
                            ap=[[0, 128], [1, D]])
            nc.gpsimd.dma_start(out=dst, in_=bcast)

        # pad-mask exp bias: (mask-1)*30 + ln(PS); pads -> exp == 0 in fp8
        kmi = const.tile([128, KT], i32, tag="kmi")
        kmf = const.tile([128, KT], f32, tag="kmf")
        padb = const.tile([128, KT], f32, tag="padb")

        # ---- long-lived activations
        kTp = ctx.enter_context(tc.tile_pool(name="kTp", bufs=1))
        kT = kTp.tile([128, KD, NK], fp8, tag="kT")
        qTp = ctx.enter_context(tc.tile_pool(name="qTp", bufs=1))
        qT = qTp.tile([128, KD, Q], fp8, tag="qT")
        vvp = ctx.enter_context(tc.tile_pool(name="vvp", bufs=1))
        vvo = vvp.tile([128, KT, 12, 68], fp8, tag="vvo")
        wop = ctx.enter_context(tc.tile_pool(name="wop", bufs=1))
        woT_sb = wop.tile([128, KD, D], fp8, tag="woT")
        x1p = ctx.enter_context(tc.tile_pool(name="x1p", bufs=1))
        x1 = x1p.tile([128, QT, D], bf16, tag="x1")
        cTp = ctx.enter_context(tc.tile_pool(name="cTp", bufs=1))
        cT = cTp.tile([128, KD, Q], fp8, tag="cT")
        h2Tp = ctx.enter_context(tc.tile_pool(name="h2Tp", bufs=1))
        h2T = h2Tp.tile([128, KD, Q], bf16, tag="h2T")

        # main PSUM pool: pss 2x2 + pc 2x1 + misc 2x1 = 8 banks
        psum_cm = tc.tile_pool(name="psum", bufs=2, space="PSUM")
        psum = psum_cm.__enter__()

        # ones column of V (softmax denominator rides along in the matmul)
        nc.vector.memset(vvo[:, :, :, 64:65], 1.0)

        # ================= Phase A: norm1 + transpose + QKV =================
        qkv_cm = tc.tile_pool(name="wqkv", bufs=1)
        wp = qkv_cm.__enter__()
        with tc.tile_pool(name="xtp", bufs=4) as xtp, \
             tc.tile_pool(name="htp", bufs=3) as htp, \
             tc.tile_pool(name="hbp", bufs=2) as hbp, \
             tc.tile_pool(name="stats", bufs=8) as spool:

            # band-0 x loads go first so the first norms are not queued
            # behind the weight DMAs
            b0 = []
            t0_, nt_ = _bands(KT, BAND)[0]
            for t in range(t0_, t0_ + nt_):
                xt = xtp.tile([128, D], bf16, tag="xt")
                nc.sync.dma_start(xt, xk_d[ts(t, 128), :])
                b0.append(xt)

            wks = wp.tile([128, KD, D], fp8, tag="wks")
            wvs = wp.tile([128, KD, D], fp8, tag="wvs")
            wqs = wp.tile([128, KD, D], fp8, tag="wqs")
            for k in range(KD):
                nc.sync.dma_start(wks[:, k, :], wk_d[ts(k, 128), :])
            for k in range(KD):
                nc.sync.dma_start(wvs[:, k, :], wv_d[ts(k, 128), :])
            for k in range(KD):
                nc.sync.dma_start(wqs[:, k, :], wq_d[ts(k, 128), :])
            nc.sync.dma_start(bqp, bq_d[:].rearrange("(o p) -> p o", p=128))
            nc.sync.dma_start(bkp, bk_d[:].rearrange("(o p) -> p o", p=128))
            nc.sync.dma_start(b1p, b1_d[:].rearrange("(o p) -> p o", p=128))
            nc.sync.dma_start(kmi, km_d[:].rearrange("(t p) -> p t", p=128))
            nc.vector.tensor_copy(out=kmf, in_=kmi)
            nc.vector.tensor_scalar(padb, kmf, 1.0, 30.0, OP.subtract, OP.mult)
            nc.vector.tensor_scalar_add(padb, padb, LOG_PS)
            for k in range(KD):
                nc.sync.dma_start(woT_sb[:, k, :], wo_d[ts(k, 128), :])

            sched = [("k", t0, nt) for t0, nt in _bands(KT, BAND)] + \
                    [("q", t0, nt) for t0, nt in _bands(QT, BAND)]

            for bi, (side, t0, nt) in enumerate(sched):
                w = nt * 128
                hbt = hbp.tile([128, KD, 512], fp8, tag="hbt")
                x_d = xk_d if side == "k" else xq_d
                for t in range(t0, t0 + nt):
                    if bi == 0:
                        xt = b0[t - t0]
                    elif side == "k":
                        xt = xtp.tile([128, D], bf16, tag="xt")
                        nc.sync.dma_start(xt, x_d[ts(t, 128), :])
                    else:
                        xt = xtp.tile([128, D], f32, tag="xtq")
                        nc.sync.dma_start(xt, x_d[ts(t, 128), :])
                    ht = htp.tile([128, D], bf16, tag="ht")
                    norm_tile(spool, xt, scal["a1"], scal["g1"], ht)
                    loc = (t - t0) * 128
                    for a in range(2):
                        ptT = psum.tile([128, 3, 128], bf16, tag="misc")
                        for i in range(3):
                            nc.tensor.transpose(ptT[:, i, :],
                                                ht[:, ts(3 * a + i, 128)],
                                                ident)
                        # psum->SBUF copy on the idle ACT engine
                        nc.scalar.activation(
                            hbt[:, ds(3 * a, 3), ds(loc, 128)], ptT,
                            AF.Copy, bias=0.0, scale=1.0)

                if side == "k":
                    for j in range(KD):
                        ps = psum.tile([128, 512], f32, tag="misc")
                        for k in range(KD2):
                            nc.tensor.matmul(ps[:, :w],
                                             wks[:, ds(2 * k, 2), ts(j, 128)],
                                             hbt[:, ds(2 * k, 2), :w],
                                             start=(k == 0),
                                             stop=(k == KD2 - 1),
                                             perf_mode=DR)
                        nc.vector.tensor_scalar(kT[:, j, ds(t0 * 128, w)],
                                                ps[:, :w], 1.0 / WS,
                                                bkp[:, j:j + 1],
                                                OP.mult, OP.add)
                    for t in range(t0, t0 + nt):
                        loc = (t - t0) * 128
                        for hh in range(2):
                            ps = psum.tile([128, 512], f32, tag="misc")
                            for k in range(KD2):
                                nc.tensor.matmul(
                                    ps[:, :384],
                                    hbt[:, ds(2 * k, 2), ds(loc, 128)],
                                    wvs[:, ds(2 * k, 2), ts(hh, 384)],
                                    start=(k == 0), stop=(k == KD2 - 1),
                                    perf_mode=DR)
                            # vvo holds 16*(v+bv); host pre-scaled bv by 16
                            nc.vector.tensor_tensor(
                                vvo[:, t, ds(6 * hh, 6), 0:64],
                                ps[:, :384].rearrange("p (h c) -> p h c", h=6),
                                bvb[:, ts(hh, 384)].rearrange(
                                    "p (h c) -> p h c", h=6),
                                OP.add)
                else:
                    for j in range(KD):
                        ps = psum.tile([128, 512], f32, tag="misc")
                        for k in range(KD2):
                            nc.tensor.matmul(ps[:, :w],
                                             wqs[:, ds(2 * k, 2), ts(j, 128)],
                                             hbt[:, ds(2 * k, 2), :w],
                                             start=(k == 0),
                                             stop=(k == KD2 - 1),
                                             perf_mode=DR)
                        nc.vector.tensor_scalar(qT[:, j, ds(t0 * 128, w)],
                                                ps[:, :w], 1.0 / WS,
                                                bqp[:, j:j + 1],
                                                OP.mult, OP.add)
        qkv_cm.__exit__(None, None, None)  # free QKV weights

        # w1/w2 stream in during chunk-0 attention (DMA is idle there)
        w12p = ctx.enter_context(tc.tile_pool(name="w12p", bufs=1))
        w1s = w12p.tile([128, KD, DFF], bf16, tag="w1s")
        w2s = w12p.tile([128, FT, D], bf16, tag="w2s")
        for k in range(KD):
            nc.sync.dma_start(w1s[:, k, :], w1_d[ts(k, 128), :])
        for k in range(FT):
            nc.sync.dma_start(w2s[:, k, :], w2_d[ts(k, 128), :])
        h3p = ctx.enter_context(tc.tile_pool(name="h3p", bufs=1))
        h3 = h3p.tile([128, FT, 512], bf16, tag="h3")

        # ============== Phase B: attention + W_O + norm2 + FFN ==============
        NPAIR = (KT + 1) // 2

        with tc.tile_pool(name="ptp", bufs=2) as ptp, \
             tc.tile_pool(name="hsp", bufs=1) as hsp, \
             tc.tile_pool(name="h2sp", bufs=2) as h2sp, \
             tc.tile_pool(name="asm", bufs=8) as asm, \
             tc.tile_pool(name="stats2", bufs=8) as spool2, \
             tc.tile_pool(name="xqb", bufs=4) as xqbp, \
             tc.tile_pool(name="outp", bufs=2) as outp:

            hstage = hsp.tile([128, 4, D], bf16, tag="hstage")

            def scores_exp(c, p, kt, ptt, parity):
                pss = psum.tile([128, 1024], f32, tag="pss")
                nc.tensor.matmul(pss[:, 0:512], kT[0:64, p, ts(kt, 128)],
                                 qT[0:64, p, ts(c, 512)], start=True, stop=True)
                nc.tensor.matmul(pss[:, 512:1024], kT[64:128, p, ts(kt, 128)],
                                 qT[64:128, p, ts(c, 512)], start=True,
                                 stop=True)
                nc.scalar.activation(ptt[:, parity, :], pss, AF.Exp,
                                     bias=padb[:, kt:kt + 1], scale=0.125)

            def divide_out(p, pc0, pc1):
                # hstage = CS * ctx = (CS/WS) * pc[:,0:64] / pc[:,64]
                for hh, pc in ((0, pc0), (1, pc1)):
                    for qb in range(4):
                        rr = asm.tile([128, 1], f32, tag="rr")
                        nc.vector.reciprocal(rr, pc[:, qb, 64:65])
                        nc.vector.tensor_scalar(
                            hstage[:, qb, ds((2 * p + hh) * 64, 64)],
                            pc[:, qb, 0:64], rr, CS / WS, OP.mult, OP.mult)

            fillers = []

            def emit_ffn(c, pspool, pbufs, split, ptag):
                """FFN closures for chunk c. split=True slices ff2 into
                small PE groups for interleaving under attention."""
                out_closures = []

                def ff1(f):
                    def go():
                        ps = pspool.tile([128, 512], f32, tag=ptag,
                                         bufs=pbufs, name="psff1")
                        for k in range(KD):
                            nc.tensor.matmul(ps, w1s[:, k, ts(f, 128)],
                                             h2T[:, k, ts(c, 512)],
                                             start=(k == 0),
                                             stop=(k == KD - 1))
                        nc.scalar.activation(h3[:, f, :], ps, AF.Relu,
                                             bias=b1p[:, f:f + 1], scale=1.0)
                    return go

                def ff2_parts(st_, hh):
                    g = c * 4 + st_
                    cell = {}

                    def part(k0, k1):
                        def go():
                            if k0 == 0:
                                cell["ps"] = pspool.tile(
                                    [128, 512], f32, tag=ptag, bufs=pbufs,
                                    name="psff2")
                            ps = cell["ps"]
                            for k in range(k0, k1):
                                nc.tensor.matmul(ps[:, :384],
                                                 h3[:, k, ts(st_, 128)],
                                                 w2s[:, k, ts(hh, 384)],
                                                 start=(k == 0),
                                                 stop=(k == FT - 1))
                            if k1 == FT:
                                ot = outp.tile([128, D], f32, tag="ot",
                                               name="otff") \
                                    if hh == 0 else ots[g]
                                nc.vector.tensor_tensor(
                                    ot[:, ts(hh, 384)], ps[:, :384],
                                    x1[:, g, ts(hh, 384)], OP.add)
                                nc.vector.tensor_tensor(
                                    ot[:, ts(hh, 384)], ot[:, ts(hh, 384)],
                                    b2b[:, ts(hh, 384)], OP.add)
                                if hh == 0:
                                    ots[g] = ot
                                else:
                                    nc.sync.dma_start(out_d[ts(g, 128), :], ot)
                        return go
                    if split:
                        return [part(k0, min(k0 + 8, FT))
                                for k0 in range(0, FT, 8)]
                    return [part(0, FT)]

                for f in range(FT):
                    out_closures.append(ff1(f))
                for st_ in range(4):
                    for hh in range(2):
                        out_closures.extend(ff2_parts(st_, hh))
                return out_closures

            ots = {}

            for c in range(QC):
                # residual tiles for this chunk's W_O (gpsimd casting DMA)
                xbs = []
                for st_ in range(4):
                    g = c * 4 + st_
                    xb = xqbp.tile([128, D], bf16, tag="xb")
                    nc.gpsimd.dma_start(out=xb, in_=xq_d[ts(g, 128), :])
                    nc.gpsimd.tensor_tensor(xb, xb, bob, OP.add)
                    xbs.append(xb)

                pend = None
                for p in range(KD):
                    pc0 = psum.tile([128, 4, 65], f32, tag="pc")
                    pc1 = psum.tile([128, 4, 65], f32, tag="pc")

                    def emit_pair(i):
                        ptt = ptp.tile([128, 2, 1024], fp8, tag="pt")
                        scores_exp(c, p, 2 * i, ptt, 0)
                        if 2 * i + 1 < KT:
                            scores_exp(c, p, 2 * i + 1, ptt, 1)
                        return ptt

                    ptt_next = emit_pair(0)
                    for i in range(NPAIR):
                        ptt = ptt_next
                        if i + 1 < NPAIR:
                            ptt_next = emit_pair(i + 1)
                        if fillers:
                            fillers.pop(0)()
                        st_f, sp_f = (i == 0), (i == NPAIR - 1)
                        full = (2 * i + 1 < KT)
                        for hh, pc in ((0, pc0), (1, pc1)):
                            head = 2 * p + hh
                            for qb in range(4):
                                col = hh * 512 + qb * 128
                                if full:
                                    nc.tensor.matmul(
                                        pc[:, qb, :],
                                        ptt[:, 0:2, ds(col, 128)],
                                        vvo[:, ds(2 * i, 2), head, 0:65],
                                        start=st_f, stop=sp_f, perf_mode=DR)
                                else:
                                    nc.tensor.matmul(
                                        pc[:, qb, :],
                                        ptt[:, 0, ds(col, 128)],
                                        vvo[:, 2 * i, head, 0:65],
                                        start=st_f, stop=sp_f)
                        if pend is not None:
                            divide_out(*pend)
                            pend = None
                    pend = (p, pc0, pc1)
                divide_out(*pend)

                # drain any leftover fillers before post-loop misc psum use
                while fillers:
                    fillers.pop(0)()

                # ---- ctx transpose + W_O + residual + norm2, per q-tile ----
                for qb in range(4):
                    g = c * 4 + qb
                    for a in range(2):
                        ptT = psum.tile([128, 3, 128], bf16, tag="misc")
                        for i in range(3):
                            nc.tensor.transpose(
                                ptT[:, i, :],
                                hstage[:, qb, ts(3 * a + i, 128)], ident)
                        nc.vector.tensor_copy(
                            out=cT[:, ds(3 * a, 3), ts(g, 128)], in_=ptT)
                    for hh in range(2):
                        ps = psum.tile([128, 512], f32, tag="misc")
                        for k in range(KD2):
                            nc.tensor.matmul(ps[:, :384],
                                             cT[:, ds(2 * k, 2), ts(g, 128)],
                                             woT_sb[:, ds(2 * k, 2),
                                                    ts(hh, 384)],
                                             start=(k == 0),
                                             stop=(k == KD2 - 1),
                                             perf_mode=DR)
                        # x1 = psum/(CS*WS) + (x + bo)
                        nc.vector.tensor_scalar(x1[:, g, ts(hh, 384)],
                                                ps[:, :384], 1.0 / (CS * WS),
                                                None, OP.mult)
                        nc.vector.tensor_tensor(x1[:, g, ts(hh, 384)],
                                                x1[:, g, ts(hh, 384)],
                                                xbs[qb][:, ts(hh, 384)],
                                                OP.add)
                    h2st = h2sp.tile([128, D], bf16, tag="h2st")
                    norm_tile(spool2, x1[:, g, :], scal["a2"], scal["g2"],
                              h2st)
                    for a in range(2):
                        ptT = psum.tile([128, 3, 128], bf16, tag="misc")
                        for i in range(3):
                            nc.tensor.transpose(ptT[:, i, :],
                                                h2st[:, ts(3 * a + i, 128)],
                                                ident)
                        nc.vector.tensor_copy(
                            out=h2T[:, ds(3 * a, 3), ts(g, 128)], in_=ptT)

                if c == 0:
                    fillers.extend(emit_ffn(0, psum, 2, split=True,
                                            ptag="misc"))

            # chunk-1 FFN: main psum pool closed, deep-buffered pool instead
            psum_cm.__exit__(None, None, None)
            with tc.tile_pool(name="pg1", bufs=8, space="PSUM") as pg1:
                for go in emit_ffn(1, pg1, 8, split=False, ptag="pff"):
                    go()

    nc.finalize()
    return nc


def _prep_inputs(inputs):
    bf = ml_dtypes.bfloat16
    f8 = ml_dtypes.float8_e4m3
    x = np.asarray(inputs["x"], np.float32)
    mask = np.asarray(inputs["mask"], np.int32).reshape(B, S)

    kept = [np.nonzero(mask[b])[0] for b in range(B)]
    nk_max = max(len(kept[0]), len(kept[1]))
    KT = max(2, int(math.ceil(nk_max / 128.0)))
    SAFE = min(len(kept[0]), len(kept[1])) // 128
    NK = KT * 128

    xk = []
    km = []
    for b in range(B):
        n = len(kept[b])
        xkb = np.zeros((NK, D), np.float32)
        xkb[:n] = x[b][kept[b]]
        if n < NK:
            # pad rows get real data (not zeros) so the norm stays finite;
            # their attention weight underflows to exactly 0 in fp8
            xkb[n:] = xkb[0]
        xk.append(np.ascontiguousarray(xkb.astype(bf)))
        kmb = np.zeros(NK, np.int32)
        kmb[:n] = 1
        km.append(kmb)

    def w_t8(name):
        return np.ascontiguousarray(
            (np.asarray(inputs[name], np.float32).T * WS).astype(f8))

    def w_tb(name):
        return np.ascontiguousarray(
            np.asarray(inputs[name], np.float32).T.astype(bf))

    shared = {
        "wqT": w_t8("wq"), "wkT": w_t8("wk"), "wvT": w_t8("wv"),
        "woT": w_t8("wo"), "w1T": w_tb("w1"), "w2T": w_tb("w2"),
        "bq": np.asarray(inputs["bq"], np.float32),
        "bk": np.asarray(inputs["bk"], np.float32),
        "bv16": np.asarray(inputs["bv"], np.float32) * WS,
        "bo": np.asarray(inputs["bo"], np.float32),
        "b1": np.asarray(inputs["b1"], np.float32),
        "b2": np.asarray(inputs["b2"], np.float32),
        "a1": np.asarray(inputs["alpha1"], np.float32).reshape(1),
        "g1": np.asarray(inputs["beta1"], np.float32).reshape(1),
        "a2": np.asarray(inputs["alpha2"], np.float32).reshape(1),
        "g2": np.asarray(inputs["beta2"], np.float32).reshape(1),
    }

    in_maps = []
    for c in range(NCORES):
        b, r = c // 4, (c % 4) * Q
        m = dict(shared)
        m["xq"] = np.ascontiguousarray(x[b, r:r + Q])
        m["xk"] = xk[b]
        m["kmask"] = km[b]
        in_maps.append(m)
    return KT, SAFE, in_maps


def kernel(**inputs):
    from concourse.bass_utils import run_bass_kernel_spmd

    KT, SAFE, in_maps = _prep_inputs(inputs)
    nc = _build(KT, SAFE)
    res = run_bass_kernel_spmd(nc, in_maps, core_ids=list(range(NCORES)))
    out = np.empty((B, S, D), np.float32)
    for c in range(NCORES):
        b, r = c // 4, (c % 4) * Q
        out[b, r:r + Q] = res.results[c]["out"]
    return out


# revision 21
# speedup vs baseline: 1.2209x; 1.0184x over previous
"""Trainium2 Bass kernel for nn_EncoderBlock (pre-norm transformer encoder).

Sharding (8 cores, zero collectives): core c -> batch b = c//4, query-row
block r = (c%4)*1024 .. +1024.  Each core redundantly computes K/V for its
batch over ONLY the keys the attention mask keeps (mask==0 keys are dropped
host-side; exp(-1e9) = 0 in the reference so they contribute nothing).

Pipeline (per core):
  norm1 -> PE-transpose -> fp8 DoubleRow Q/K/V projections (weights x16 fp8)
  scores^T = K^T.T @ Q^T in fp8 operands / f32 psum
  P' = exp(scores/8 + padbias + ln PS) -> fp8 (pads underflow to exact 0)
  ctx[q,d] accumulated with queries on PSUM partitions; softmax denominator
    rides as a 65th ones-column of V (fused into the same matmul)
  divide (per-partition scalar) -> PE-transpose ctx -> W_O fp8-DR -> x1
  norm2 -> PE-transpose -> FFN in bf16 (fp8 fails the accuracy budget here);
  chunk-0 FFN is interleaved into chunk-1's attention stream in small slices
  to keep PE dense under the ACT-bound softmax phase.
"""

import math
from contextlib import ExitStack

import ml_dtypes
import numpy as np

B, S, D = 2, 4096, 768
H, DK, DFF = 12, 64, 3072
KD = D // 128         # 6 k-tiles over d_model
KD2 = KD // 2         # 3 DoubleRow steps over d_model
FT = DFF // 128       # 24 tiles over d_ff
Q = 1024              # query rows per core
QT = Q // 128
QC = 2                # q chunks of 512
NCORES = 8
EPS = 1e-6
VAR_SCALE = float(D) / float(D - 1)
BAND = 4
WS = 16.0             # host-side fp8 weight scale (qkv/wo only)
CS = 16.0             # ctx scale in hstage/cT (fp8 range)
PS = 0.25             # P' = PS * softmax numerator (fp8 range, no overflow)
LOG_PS = math.log(PS)

DEBUG = False


def _bands(ntiles, band):
    out = []
    t = 0
    while t < ntiles:
        out.append((t, min(band, ntiles - t)))
        t += band
    return out


def _build(KT, SAFE=None):
    import concourse.bass as bass
    import concourse.mybir as mybir
    import concourse.tile as tile
    from concourse import bacc
    from concourse.bass import ds, ts

    NK = KT * 128
    f32 = mybir.dt.float32
    bf16 = mybir.dt.bfloat16
    fp8 = mybir.dt.float8e4
    i32 = mybir.dt.int32
    AF = mybir.ActivationFunctionType
    OP = mybir.AluOpType
    DR = mybir.MatmulPerfMode.DoubleRow

    nc = bacc.Bacc()

    xq_d = nc.dram_tensor("xq", [Q, D], f32, kind="ExternalInput")
    xk_d = nc.dram_tensor("xk", [NK, D], bf16, kind="ExternalInput")
    km_d = nc.dram_tensor("kmask", [NK], i32, kind="ExternalInput")
    wq_d = nc.dram_tensor("wqT", [D, D], fp8, kind="ExternalInput")
    wk_d = nc.dram_tensor("wkT", [D, D], fp8, kind="ExternalInput")
    wv_d = nc.dram_tensor("wvT", [D, D], fp8, kind="ExternalInput")
    wo_d = nc.dram_tensor("woT", [D, D], fp8, kind="ExternalInput")
    w1_d = nc.dram_tensor("w1T", [D, DFF], bf16, kind="ExternalInput")
    w2_d = nc.dram_tensor("w2T", [DFF, D], bf16, kind="ExternalInput")
    bq_d = nc.dram_tensor("bq", [D], f32, kind="ExternalInput")
    bk_d = nc.dram_tensor("bk", [D], f32, kind="ExternalInput")
    bv_d = nc.dram_tensor("bv16", [D], f32, kind="ExternalInput")
    bo_d = nc.dram_tensor("bo", [D], f32, kind="ExternalInput")
    b1_d = nc.dram_tensor("b1", [DFF], f32, kind="ExternalInput")
    b2_d = nc.dram_tensor("b2", [D], f32, kind="ExternalInput")
    a1_d = nc.dram_tensor("a1", [1], f32, kind="ExternalInput")
    g1_d = nc.dram_tensor("g1", [1], f32, kind="ExternalInput")
    a2_d = nc.dram_tensor("a2", [1], f32, kind="ExternalInput")
    g2_d = nc.dram_tensor("g2", [1], f32, kind="ExternalInput")
    out_d = nc.dram_tensor("out", [Q, D], f32, kind="ExternalOutput")

    def norm_tile(spool, xt, a_b, g_b, out_t):
        # out = alpha * (x - mean) / (std_unbiased + eps) + beta over D
        st = spool.tile([128, 3, 6], f32, tag="bnst")
        for g in range(3):
            nc.vector.bn_stats(st[:, g, :], xt[:, ts(g, 256)])
        mv = spool.tile([128, 2], f32, tag="bnmv")
        nc.vector.bn_aggr(mv, st)
        rp = spool.tile([128, 1], f32, tag="rp")
        nc.scalar.activation(rp, mv[:, 1:2], AF.Sqrt, bias=0.0, scale=VAR_SCALE)
        nc.vector.reciprocal(rp, rp)
        nc.vector.tensor_tensor(rp, rp, a_b, OP.mult)
        cb = spool.tile([128, 1], f32, tag="cb")
        nc.vector.tensor_tensor(cb, mv[:, 0:1], rp, OP.mult)
        nc.vector.tensor_tensor(cb, g_b, cb, OP.subtract)
        nc.gpsimd.tensor_scalar(out_t, xt, rp, cb, OP.mult, OP.add)

    with tile.TileContext(nc) as tc, ExitStack() as ctx:
        const = ctx.enter_context(tc.tile_pool(name="const", bufs=1))

        scal = {}
        for name, d_t in (("a1", a1_d), ("g1", g1_d), ("a2", a2_d), ("g2", g2_d)):
            t = const.tile([128, 1], f32, tag=f"sc_{name}")
            nc.gpsimd.dma_start(out=t, in_=d_t[:].to_broadcast((128, 1)))
            scal[name] = t

        bqp = const.tile([128, KD], f32, tag="bqp")
        bkp = const.tile([128, KD], f32, tag="bkp")
        b1p = const.tile([128, FT], f32, tag="b1p")

        # warm the sqrt activation table before the first norm needs it
        warmt = const.tile([128, 1], f32, tag="warmt")
        nc.vector.memset(warmt, 1.0)
        nc.scalar.activation(warmt, warmt, AF.Sqrt, bias=0.0, scale=1.0)
        ident = const.tile([128, 128], bf16, tag="ident")
        from concourse.masks import make_identity
        make_identity(nc, ident)

        bvb = const.tile([128, D], f32, tag="bvb")   # 16*bv broadcast
        bob = const.tile([128, D], f32, tag="bob")
        b2b = const.tile([128, D], f32, tag="b2b")
        for d_t, dst in ((bv_d, bvb), (bo_d, bob), (b2_d, b2b)):
            src = d_t[:]
            bcast = bass.AP(tensor=src.tensor, offset=src.offset,
                            ap=[[0, 128], [1, D]])
            nc.gpsimd.dma_start(out=dst, in_=bcast)

        # pad-mask exp bias: (mask-1)*30 + ln(PS); pads -> exp == 0 in fp8
        kmi = const.tile([128, KT], i32, tag="kmi")
        kmf = const.tile([128, KT], f32, tag="kmf")
        padb = const.tile([128, KT], f32, tag="padb")

        # ---- long-lived activations
        kTp = ctx.enter_context(tc.tile_pool(name="kTp", bufs=1))
        kT = kTp.tile([128, KD, NK], fp8, tag="kT")
        qTp = ctx.enter_context(tc.tile_pool(name="qTp", bufs=1))
        qT = qTp.tile([128, KD, Q], fp8, tag="qT")
        vvp = ctx.enter_context(tc.tile_pool(name="vvp", bufs=1))
        vvo = vvp.tile([128, KT, 12, 68], fp8, tag="vvo")
        wop = ctx.enter_context(tc.tile_pool(name="wop", bufs=1))
        woT_sb = wop.tile([128, KD, D], fp8, tag="woT")
        x1p = ctx.enter_context(tc.tile_pool(name="x1p", bufs=1))
        x1 = x1p.tile([128, QT, D], bf16, tag="x1")
        cTp = ctx.enter_context(tc.tile_pool(name="cTp", bufs=1))
        cT = cTp.tile([128, KD, Q], fp8, tag="cT")
        h2Tp = ctx.enter_context(tc.tile_pool(name="h2Tp", bufs=1))
        h2T = h2Tp.tile([128, KD, Q], bf16, tag="h2T")

        # main PSUM pool: pss 2x2 + pc 2x1 + misc 2x1 = 8 banks
        psum_cm = tc.tile_pool(name="psum", bufs=2, space="PSUM")
        psum = psum_cm.__enter__()

        # ones column of V (softmax denominator rides along in the matmul)
        nc.vector.memset(vvo[:, :, :, 64:65], 1.0)

        # ================= Phase A: norm1 + transpose + QKV =================
        qkv_cm = tc.tile_pool(name="wqkv", bufs=1)
        wp = qkv_cm.__enter__()
        with tc.tile_pool(name="xtp", bufs=4) as xtp, \
             tc.tile_pool(name="htp", bufs=3) as htp, \
             tc.tile_pool(name="hbp", bufs=2) as hbp, \
             tc.tile_pool(name="stats", bufs=8) as spool:

            # band-0 x loads go first so the first norms are not queued
            # behind the weight DMAs
            b0 = []
            t0_, nt_ = _bands(KT, BAND)[0]
            for t in range(t0_, t0_ + nt_):
                xt = xtp.tile([128, D], bf16, tag="xt")
                nc.sync.dma_start(xt, xk_d[ts(t, 128), :])
                b0.append(xt)

            wks = wp.tile([128, KD, D], fp8, tag="wks")
            wvs = wp.tile([128, KD, D], fp8, tag="wvs")
            wqs = wp.tile([128, KD, D], fp8, tag="wqs")
            for k in range(KD):
                nc.sync.dma_start(wks[:, k, :], wk_d[ts(k, 128), :])
            for k in range(KD):
                nc.sync.dma_start(wvs[:, k, :], wv_d[ts(k, 128), :])
            for k in range(KD):
                nc.sync.dma_start(wqs[:, k, :], wq_d[ts(k, 128), :])
            nc.sync.dma_start(bqp, bq_d[:].rearrange("(o p) -> p o", p=128))
            nc.sync.dma_start(bkp, bk_d[:].rearrange("(o p) -> p o", p=128))
            nc.sync.dma_start(b1p, b1_d[:].rearrange("(o p) -> p o", p=128))
            nc.sync.dma_start(kmi, km_d[:].rearrange("(t p) -> p t", p=128))
            nc.vector.tensor_copy(out=kmf, in_=kmi)
            nc.vector.tensor_scalar(padb, kmf, 1.0, 30.0, OP.subtract, OP.mult)
            nc.vector.tensor_scalar_add(padb, padb, LOG_PS)
            for k in range(KD):
                nc.sync.dma_start(woT_sb[:, k, :], wo_d[ts(k, 128), :])

            sched = [("k", t0, nt) for t0, nt in _bands(KT, BAND)] + \
                    [("q", t0, nt) for t0, nt in _bands(QT, BAND)]

            for bi, (side, t0, nt) in enumerate(sched):
                w = nt * 128
                hbt = hbp.tile([128, KD, 512], fp8, tag="hbt")
                x_d = xk_d if side == "k" else xq_d
                for t in range(t0, t0 + nt):
                    if bi == 0:
                        xt = b0[t - t0]
                    elif side == "k":
                        xt = xtp.tile([128, D], bf16, tag="xt")
                        nc.sync.dma_start(xt, x_d[ts(t, 128), :])
                    else:
                        xt = xtp.tile([128, D], f32, tag="xtq")
                        nc.sync.dma_start(xt, x_d[ts(t, 128), :])
                    ht = htp.tile([128, D], bf16, tag="ht")
                    norm_tile(spool, xt, scal["a1"], scal["g1"], ht)
                    loc = (t - t0) * 128
                    for a in range(2):
                        ptT = psum.tile([128, 3, 128], bf16, tag="misc")
                        for i in range(3):
                            nc.tensor.transpose(ptT[:, i, :],
                                                ht[:, ts(3 * a + i, 128)],
                                                ident)
                        # psum->SBUF copy on the idle ACT engine
                        nc.scalar.activation(
                            hbt[:, ds(3 * a, 3), ds(loc, 128)], ptT,
                            AF.Copy, bias=0.0, scale=1.0)

                if side == "k":
                    for j in range(KD):
                        ps = psum.tile([128, 512], f32, tag="misc")
                        for k in range(KD2):
                            nc.tensor.matmul(ps[:, :w],
                                             wks[:, ds(2 * k, 2), ts(j, 128)],
                                             hbt[:, ds(2 * k, 2), :w],
                                             start=(k == 0),
                                             stop=(k == KD2 - 1),
                                             perf_mode=DR)
                        nc.scalar.activation(kT[:, j, ds(t0 * 128, w)],
                                             ps[:, :w], AF.Identity,
                                             bias=bkp[:, j:j + 1],
                                             scale=1.0 / WS)
                    for t in range(t0, t0 + nt):
                        loc = (t - t0) * 128
                        for hh in range(2):
                            ps = psum.tile([128, 512], f32, tag="misc")
                            for k in range(KD2):
                                nc.tensor.matmul(
                                    ps[:, :384],
                                    hbt[:, ds(2 * k, 2), ds(loc, 128)],
                                    wvs[:, ds(2 * k, 2), ts(hh, 384)],
                                    start=(k == 0), stop=(k == KD2 - 1),
                                    perf_mode=DR)
                            # vvo holds 16*(v+bv); host pre-scaled bv by 16
                            nc.vector.tensor_tensor(
                                vvo[:, t, ds(6 * hh, 6), 0:64],
                                ps[:, :384].rearrange("p (h c) -> p h c", h=6),
                                bvb[:, ts(hh, 384)].rearrange(
                                    "p (h c) -> p h c", h=6),
                                OP.add)
                else:
                    for j in range(KD):
                        ps = psum.tile([128, 512], f32, tag="misc")
                        for k in range(KD2):
                            nc.tensor.matmul(ps[:, :w],
                                             wqs[:, ds(2 * k, 2), ts(j, 128)],
                                             hbt[:, ds(2 * k, 2), :w],
                                             start=(k == 0),
                                             stop=(k == KD2 - 1),
                                             perf_mode=DR)
                        nc.scalar.activation(qT[:, j, ds(t0 * 128, w)],
                                             ps[:, :w], AF.Identity,
                                             bias=bqp[:, j:j + 1],
                                             scale=1.0 / WS)
        qkv_cm.__exit__(None, None, None)  # free QKV weights

        # w1/w2 stream in during chunk-0 attention (DMA is idle there)
        w12p = ctx.enter_context(tc.tile_pool(name="w12p", bufs=1))
        w1s = w12p.tile([128, KD, DFF], bf16, tag="w1s")
        w2s = w12p.tile([128, FT, D], bf16, tag="w2s")
        for k in range(KD):
            nc.sync.dma_start(w1s[:, k, :], w1_d[ts(k, 128), :])
        for k in range(FT):
            nc.sync.dma_start(w2s[:, k, :], w2_d[ts(k, 128), :])
        h3p = ctx.enter_context(tc.tile_pool(name="h3p", bufs=1))
        h3 = h3p.tile([128, FT, 512], bf16, tag="h3")

        # ============== Phase B: attention + W_O + norm2 + FFN ==============
        NPAIR = (KT + 1) // 2

        with tc.tile_pool(name="ptp", bufs=2) as ptp, \
             tc.tile_pool(name="hsp", bufs=2) as hsp, \
             tc.tile_pool(name="h2sp", bufs=2) as h2sp, \
             tc.tile_pool(name="asm", bufs=8) as asm, \
             tc.tile_pool(name="stats2", bufs=8) as spool2, \
             tc.tile_pool(name="xqb", bufs=2) as xqbp, \
             tc.tile_pool(name="outp", bufs=2) as outp:

            def scores_exp(c, p, kt, ptt, parity):
                pss = psum.tile([128, 1024], f32, tag="pss")
                nc.tensor.matmul(pss[:, 0:512], kT[0:64, p, ts(kt, 128)],
                                 qT[0:64, p, ts(c, 512)], start=True, stop=True)
                nc.tensor.matmul(pss[:, 512:1024], kT[64:128, p, ts(kt, 128)],
                                 qT[64:128, p, ts(c, 512)], start=True,
                                 stop=True)
                nc.scalar.activation(ptt[:, parity, :], pss, AF.Exp,
                                     bias=padb[:, kt:kt + 1], scale=0.125)

            def divide_out(p, pc0, pc1, hstage):
                # hstage = CS * ctx = (CS/WS) * pc[:,0:64] / pc[:,64]
                for hh, pc in ((0, pc0), (1, pc1)):
                    for qb in range(4):
                        rr = asm.tile([128, 1], f32, tag="rr")
                        nc.vector.reciprocal(rr, pc[:, qb, 64:65])
                        nc.vector.tensor_scalar(
                            hstage[:, qb, ds((2 * p + hh) * 64, 64)],
                            pc[:, qb, 0:64], rr, CS / WS, OP.mult, OP.mult)

            def post_qtile(c, qb, hstage, pspool, pbufs, ptag):
                # ctx transpose + W_O + residual + norm2 + h2 transpose
                def go():
                    g = c * 4 + qb
                    xb = xqbp.tile([128, D], bf16, tag=f"xb{c}")
                    nc.gpsimd.dma_start(out=xb, in_=xq_d[ts(g, 128), :])
                    nc.gpsimd.tensor_tensor(xb, xb, bob, OP.add)
                    for a in range(2):
                        ptT = pspool.tile([128, 3, 128], bf16, tag=ptag,
                                          bufs=pbufs, name="ptT")
                        for i in range(3):
                            nc.tensor.transpose(
                                ptT[:, i, :],
                                hstage[:, qb, ts(3 * a + i, 128)], ident)
                        nc.vector.tensor_copy(
                            out=cT[:, ds(3 * a, 3), ts(g, 128)], in_=ptT)
                    for hh in range(2):
                        ps = pspool.tile([128, 512], f32, tag=ptag,
                                         bufs=pbufs, name="pswo")
                        for k in range(KD2):
                            nc.tensor.matmul(ps[:, :384],
                                             cT[:, ds(2 * k, 2), ts(g, 128)],
                                             woT_sb[:, ds(2 * k, 2),
                                                    ts(hh, 384)],
                                             start=(k == 0),
                                             stop=(k == KD2 - 1),
                                             perf_mode=DR)
                        # x1 = psum/(CS*WS) + (x + bo)
                        nc.vector.tensor_scalar(x1[:, g, ts(hh, 384)],
                                                ps[:, :384], 1.0 / (CS * WS),
                                                None, OP.mult)
                        nc.vector.tensor_tensor(x1[:, g, ts(hh, 384)],
                                                x1[:, g, ts(hh, 384)],
                                                xb[:, ts(hh, 384)],
                                                OP.add)
                    h2st = h2sp.tile([128, D], bf16, tag="h2st")
                    norm_tile(spool2, x1[:, g, :], scal["a2"], scal["g2"],
                              h2st)
                    for a in range(2):
                        ptT = pspool.tile([128, 3, 128], bf16, tag=ptag,
                                          bufs=pbufs, name="ptT2")
                        for i in range(3):
                            nc.tensor.transpose(ptT[:, i, :],
                                                h2st[:, ts(3 * a + i, 128)],
                                                ident)
                        nc.vector.tensor_copy(
                            out=h2T[:, ds(3 * a, 3), ts(g, 128)], in_=ptT)
                return go

            fillers = []

            def emit_ffn(c, pspool, pbufs, split, ptag):
                """FFN closures for chunk c. split=True slices ff2 into
                small PE groups for interleaving under attention."""
                out_closures = []

                def ff1(f):
                    def go():
                        ps = pspool.tile([128, 512], f32, tag=ptag,
                                         bufs=pbufs, name="psff1")
                        for k in range(KD):
                            nc.tensor.matmul(ps, w1s[:, k, ts(f, 128)],
                                             h2T[:, k, ts(c, 512)],
                                             start=(k == 0),
                                             stop=(k == KD - 1))
                        nc.scalar.activation(h3[:, f, :], ps, AF.Relu,
                                             bias=b1p[:, f:f + 1], scale=1.0)
                    return go

                def ff2_parts(st_, hh):
                    g = c * 4 + st_
                    cell = {}

                    def part(k0, k1):
                        def go():
                            if k0 == 0:
                                cell["ps"] = pspool.tile(
                                    [128, 512], f32, tag=ptag, bufs=pbufs,
                                    name="psff2")
                            ps = cell["ps"]
                            for k in range(k0, k1):
                                nc.tensor.matmul(ps[:, :384],
                                                 h3[:, k, ts(st_, 128)],
                                                 w2s[:, k, ts(hh, 384)],
                                                 start=(k == 0),
                                                 stop=(k == FT - 1))
                            if k1 == FT:
                                ot = outp.tile([128, D], f32, tag="ot",
                                               name="otff") \
                                    if hh == 0 else ots[g]
                                nc.vector.tensor_tensor(
                                    ot[:, ts(hh, 384)], ps[:, :384],
                                    x1[:, g, ts(hh, 384)], OP.add)
                                nc.vector.tensor_tensor(
                                    ot[:, ts(hh, 384)], ot[:, ts(hh, 384)],
                                    b2b[:, ts(hh, 384)], OP.add)
                                if hh == 0:
                                    ots[g] = ot
                                else:
                                    nc.sync.dma_start(out_d[ts(g, 128), :], ot)
                        return go
                    if split:
                        return [part(k0, min(k0 + 8, FT))
                                for k0 in range(0, FT, 8)]
                    return [part(0, FT)]

                for f in range(FT):
                    out_closures.append(ff1(f))
                for st_ in range(4):
                    for hh in range(2):
                        out_closures.extend(ff2_parts(st_, hh))
                return out_closures

            ots = {}

            for c in range(QC):
                hstage = hsp.tile([128, 4, D], bf16, tag="hstage")

                pend = None
                for p in range(KD):
                    pc0 = psum.tile([128, 4, 65], f32, tag="pc")
                    pc1 = psum.tile([128, 4, 65], f32, tag="pc")

                    def emit_pair(i):
                        ptt = ptp.tile([128, 2, 1024], fp8, tag="pt")
                        scores_exp(c, p, 2 * i, ptt, 0)
                        if 2 * i + 1 < KT:
                            scores_exp(c, p, 2 * i + 1, ptt, 1)
                        return ptt

                    ptt_next = emit_pair(0)
                    for i in range(NPAIR):
                        ptt = ptt_next
                        if i + 1 < NPAIR:
                            ptt_next = emit_pair(i + 1)
                        if fillers:
                            fillers.pop(0)()
                        st_f, sp_f = (i == 0), (i == NPAIR - 1)
                        full = (2 * i + 1 < KT)
                        for hh, pc in ((0, pc0), (1, pc1)):
                            head = 2 * p + hh
                            for qb in range(4):
                                col = hh * 512 + qb * 128
                                if full:
                                    nc.tensor.matmul(
                                        pc[:, qb, :],
                                        ptt[:, 0:2, ds(col, 128)],
                                        vvo[:, ds(2 * i, 2), head, 0:65],
                                        start=st_f, stop=sp_f, perf_mode=DR)
                                else:
                                    nc.tensor.matmul(
                                        pc[:, qb, :],
                                        ptt[:, 0, ds(col, 128)],
                                        vvo[:, 2 * i, head, 0:65],
                                        start=st_f, stop=sp_f)
                        if pend is not None:
                            divide_out(*pend)
                            pend = None
                    pend = (p, pc0, pc1, hstage)
                divide_out(*pend)

                # drain any leftover fillers before post-loop misc psum use
                while fillers:
                    fillers.pop(0)()

                if c == 0:
                    # chunk-0 post-processing + FFN run as fillers inside
                    # chunk-1's attention stream to keep PE dense there
                    fillers.extend(
                        [post_qtile(0, qb, hstage, psum, 2, "misc")
                         for qb in range(4)] +
                        emit_ffn(0, psum, 2, split=True, ptag="misc"))
                else:
                    # main psum pool closed; deep-buffered pool for the tail
                    psum_cm.__exit__(None, None, None)
                    with tc.tile_pool(name="pg1", bufs=8, space="PSUM") as pg1:
                        for qb in range(4):
                            post_qtile(1, qb, hstage, pg1, 8, "pff")()
                        for go in emit_ffn(1, pg1, 8, split=False,
                                           ptag="pff"):
                            go()

    nc.finalize()
    return nc


def _prep_inputs(inputs):
    bf = ml_dtypes.bfloat16
    f8 = ml_dtypes.float8_e4m3
    x = np.asarray(inputs["x"], np.float32)
    mask = np.asarray(inputs["mask"], np.int32).reshape(B, S)

    kept = [np.nonzero(mask[b])[0] for b in range(B)]
    nk_max = max(len(kept[0]), len(kept[1]))
    KT = max(2, int(math.ceil(nk_max / 128.0)))
    SAFE = min(len(kept[0]), len(kept[1])) // 128
    NK = KT * 128

    xk = []
    km = []
    for b in range(B):
        n = len(kept[b])
        xkb = np.zeros((NK, D), np.float32)
        xkb[:n] = x[b][kept[b]]
        if n < NK:
            # pad rows get real data (not zeros) so the norm stays finite;
            # their attention weight underflows to exactly 0 in fp8
            xkb[n:] = xkb[0]
        xk.append(np.ascontiguousarray(xkb.astype(bf)))
        kmb = np.zeros(NK, np.int32)
        kmb[:n] = 1
        km.append(kmb)

    def w_t8(name):
        return np.ascontiguousarray(
            (np.asarray(inputs[name], np.float32).T * WS).astype(f8))

    def w_tb(name):
        return np.ascontiguousarray(
            np.asarray(inputs[name], np.float32).T.astype(bf))

    shared = {
        "wqT": w_t8("wq"), "wkT": w_t8("wk"), "wvT": w_t8("wv"),
        "woT": w_t8("wo"), "w1T": w_tb("w1"), "w2T": w_tb("w2"),
        "bq": np.asarray(inputs["bq"], np.float32),
        "bk": np.asarray(inputs["bk"], np.float32),
        "bv16": np.asarray(inputs["bv"], np.float32) * WS,
        "bo": np.asarray(inputs["bo"], np.float32),
        "b1": np.asarray(inputs["b1"], np.float32),
        "b2": np.asarray(inputs["b2"], np.float32),
        "a1": np.asarray(inputs["alpha1"], np.float32).reshape(1),
        "g1": np.asarray(inputs["beta1"], np.float32).reshape(1),
        "a2": np.asarray(inputs["alpha2"], np.float32).reshape(1),
        "g2": np.asarray(inputs["beta2"], np.float32).reshape(1),
    }

    in_maps = []
    for c in range(NCORES):
        b, r = c // 4, (c % 4) * Q
        m = dict(shared)
        m["xq"] = np.ascontiguousarray(x[b, r:r + Q])
        m["xk"] = xk[b]
        m["kmask"] = km[b]
        in_maps.append(m)
    return KT, SAFE, in_maps


def kernel(**inputs):
    from concourse.bass_utils import run_bass_kernel_spmd

    KT, SAFE, in_maps = _prep_inputs(inputs)
    nc = _build(KT, SAFE)
    res = run_bass_kernel_spmd(nc, in_maps, core_ids=list(range(NCORES)))
    out = np.empty((B, S, D), np.float32)
    for c in range(NCORES):
        b, r = c // 4, (c % 4) * Q
        out[b, r:r + Q] = res.results[c]["out"]
    return out
